# revision 26
# baseline (speedup 1.0000x reference)
"""Trainium2 Bass kernel for nn_Decoder (GRU decoder + MLP + vocab softmax).

Sharding (8 NeuronCores):
  - GRU + 2-layer MLP: data-parallel over batch (4 examples/core).
    Local tokens are b-major (col = b*128 + t) so the global token index
    G = 512*rank + b*128 + t equals example*128 + t, matching output rows.
  - h2^T (fp8 x16) all-gathered across cores in 6 step-chunks DURING the
    GRU scan; vocab tiles cover (step-chunk x ranks) so the [512,32000]
    column-parallel projection + softmax is interleaved into the GRU's
    engine gaps instead of running as a separate phase.

Precision: recurrent kernel fp8e4 (x32), vocab matmul fp8xfp8 DoubleRow
(h2 x16, w3 x64), else bf16 operands / fp32 PSUM.  Output bf16, upcast on
host.  ~1.3e-2 rel err vs fp32 reference (gate 2e-2).

GRU gates use Tanh (same ScalarE table set as Exp, so interleaving vocab
exp causes no ACT table reloads): sigmoid(x) = (tanh(x/2)+1)/2, folded
into a 5-op scalar_tensor_tensor chain on VectorE:
    u  = (r' + 1) * h_ps            # = 64 * r * rh
    w  = (u / 64) + xh
    d  = max(w, 0) - h_prev         # = hh - h_prev
    f  = (z' - 1) * d               # = (1-z')(h_prev-hh)
    hs = (f * -0.5) + h_prev        # = z*h_prev + (1-z)*hh
"""

from collections import deque

import numpy as np

import concourse.bass as bass
import concourse.tile as tile
from concourse import bacc, mybir
from concourse.bass import ds, ts
from concourse.bass_utils import run_bass_kernel_spmd
from concourse.masks import make_identity

P = 128
NCORES = 8
B, T, E, H, V = 32, 128, 256, 512, 32000
BL = B // NCORES            # 4 examples per core
NTOK = BL * T               # 512 local tokens
G = B * T                   # 4096 global tokens
VS = V // NCORES            # 4000 vocab cols per core
KO = H // P                 # 4 hidden chunks
MO3 = 3 * H // P            # 12 gate chunks
SO = (E + H) // P           # 6 input chunks
NJ = 8                      # vocab col chunks per token tile (8 x 500)
VC = VS // NJ               # 500

CHS = [16, 16, 32, 32, 16, 16]          # GRU steps per gather chunk
CH_STARTS = [sum(CHS[:i]) for i in range(len(CHS))]
NCH = len(CHS)
ROUND = 4                               # tiles per softmax all-reduce round
NTILES = G // P                         # 32
NROUNDS = NTILES // ROUND               # 8
import os as _os
AG_MARGIN = int(_os.environ.get("AG_MARGIN", "14"))     # steps before a chunk's tiles unlock

SCALE_R = 32.0
SCALE_H2 = 16.0
SCALE_W3 = 64.0
INV_R = 1.0 / SCALE_R
INV_LOGIT = 1.0 / (SCALE_H2 * SCALE_W3)

f32 = mybir.dt.float32
bf16 = mybir.dt.bfloat16
fp8 = mybir.dt.float8e4

TRACE = False
TRACE_KWARGS = {}
LAST_RESULT = None

RG = [list(range(NCORES))]
DR = mybir.MatmulPerfMode.DoubleRow

Copy = mybir.ActivationFunctionType.Copy
Ident = mybir.ActivationFunctionType.Identity
Tanh = mybir.ActivationFunctionType.Tanh
Relu = mybir.ActivationFunctionType.Relu
Exp = mybir.ActivationFunctionType.Exp
Add = mybir.AluOpType.add
Sub = mybir.AluOpType.subtract
Mult = mybir.AluOpType.mult
Max = mybir.AluOpType.max


def _build(has_b3: bool, has_gb: bool):
    nc = bacc.Bacc("TRN2", target_bir_lowering=False, debug=False,
                   num_devices=NCORES)

    ext = {}
    ext["enc"] = nc.dram_tensor("encoder_input", [BL, T, E], f32, kind="ExternalInput").ap()
    ext["dec"] = nc.dram_tensor("decoder_input", [BL, H], f32, kind="ExternalInput").ap()
    ext["gk"] = nc.dram_tensor("gru_kernel", [E + H, 3 * H], f32, kind="ExternalInput").ap()
    ext["gr"] = nc.dram_tensor("gru_rec_kernel", [H, 3 * H], f32, kind="ExternalInput").ap()
    ext["gb"] = nc.dram_tensor("gru_bias", [2, 3 * H], f32, kind="ExternalInput").ap()
    ext["w1"] = nc.dram_tensor("w1", [H, H], f32, kind="ExternalInput").ap()
    ext["b1"] = nc.dram_tensor("b1", [H], f32, kind="ExternalInput").ap()
    ext["w2"] = nc.dram_tensor("w2", [H, H], f32, kind="ExternalInput").ap()
    ext["b2"] = nc.dram_tensor("b2", [H], f32, kind="ExternalInput").ap()
    ext["w3"] = nc.dram_tensor("w3", [H, VS], f32, kind="ExternalInput").ap()
    ext["b3"] = nc.dram_tensor("b3", [VS], f32, kind="ExternalInput").ap()
    ext["out"] = nc.dram_tensor("out", [G, VS], bf16, kind="ExternalOutput").ap()
    import os
    if os.environ.get("DBG_HSEQ"):
        ext["dbg"] = nc.dram_tensor("dbg", [P, KO, NTOK], f32, kind="ExternalOutput").ap()
    if os.environ.get("DBG_H2G"):
        ext["dbg2"] = nc.dram_tensor("dbg2", [P, KO, G], fp8, kind="ExternalOutput").ap()
    if os.environ.get("DBG_SUMS"):
        ext["dbg3"] = nc.dram_tensor("dbg3", [NROUNDS, 2, P, ROUND], f32, kind="ExternalOutput").ap()
    if os.environ.get("DBG_W3"):
        ext["dbg4"] = nc.dram_tensor("dbg4", [P, KO, VS], fp8, kind="ExternalOutput").ap()
    if os.environ.get("DBG_EXP"):
        ext["dbg5"] = nc.dram_tensor("dbg5", [P, NJ, VC], bf16, kind="ExternalOutput").ap()

    with tile.TileContext(nc) as tc:
        with tc.tile_pool(name="dram", bufs=1, space="DRAM") as dram_pool:
            h2_bounce = [dram_pool.tile([H, CHS[c] * BL], fp8, name=f"h2b_{c}")
                         for c in range(NCH)]
            h2_gath = [dram_pool.tile([NCORES * H, CHS[c] * BL], fp8,
                                      addr_space="Shared", name=f"h2g_{c}")
                       for c in range(NCH)]
            sums_in = [dram_pool.tile([P * ROUND], f32, name=f"sums_in_{r}")
                       for r in range(NROUNDS)]
            sums_out = [dram_pool.tile([P * ROUND], f32, addr_space="Shared",
                                       name=f"sums_out_{r}")
                        for r in range(NROUNDS)]
            _body(nc, tc, has_b3, has_gb, ext,
                  h2_bounce, h2_gath, sums_in, sums_out)
    nc.finalize()
    return nc


def _body(nc, tc, has_b3, has_gb, ext, h2_bounce, h2_gath, sums_in, sums_out):
    from contextlib import ExitStack

    stack = ExitStack()
    gpool = stack.enter_context(tc.tile_pool(name="gpool", bufs=1))
    wtmp_pool = stack.enter_context(tc.tile_pool(name="wtmp", bufs=2))
    gt_pool = stack.enter_context(tc.tile_pool(name="gt", bufs=3))
    h1_pool = stack.enter_context(tc.tile_pool(name="h1p", bufs=2))
    exp_pool = stack.enter_context(tc.tile_pool(name="exp", bufs=8))
    out_pool = stack.enter_context(tc.tile_pool(name="outp", bufs=2))
    sc_pool = stack.enter_context(tc.tile_pool(name="scp", bufs=3))
    ps_gru = stack.enter_context(tc.tile_pool(name="ps_gru", bufs=1, space="PSUM"))
    ps_pro = stack.enter_context(tc.tile_pool(name="ps_pro", bufs=2, space="PSUM"))
    ps_voc = stack.enter_context(tc.tile_pool(name="ps_voc", bufs=2, space="PSUM"))

    w3b = gpool.tile([P, KO, VS], fp8)
    b3bc = gpool.tile([P, VS], f32, name="b3bc") if has_b3 else None
    w1b = gpool.tile([P, KO, H], bf16)
    w2b = gpool.tile([P, KO, H], bf16)
    b1T = gpool.tile([P, KO], f32)
    b2T = gpool.tile([P, KO], f32)
    Rb = gpool.tile([P, KO, 3 * H], fp8)
    Wkb = gpool.tile([P, SO, 3 * H], bf16)
    seqT = gpool.tile([P, SO, NTOK], bf16)
    xprojT = gpool.tile([P, MO3, NTOK], bf16)
    hseqT = gpool.tile([P, KO, NTOK], bf16)
    h28T = gpool.tile([P, KO, NTOK], fp8)
    h2g8 = [gpool.tile([P, KO, NCORES * CHS[c] * BL], fp8, name=f"h2g8_{c}")
            for c in range(NCH)]

    # ---------------- initial loads ----------------
    seqT4 = seqT.rearrange("p so (b t) -> p so b t", b=BL)
    ident = gpool.tile([P, P], bf16)
    make_identity(nc, ident)
    enc_nat = wtmp_pool.tile([P, BL, E], f32, tag="wtmp")
    nc.sync.dma_start(out=enc_nat[:], in_=ext["enc"].rearrange("b t c -> t b c"))
    enc_natb = wtmp_pool.tile([P, BL, E], bf16, tag="encb", bufs=1)
    nc.vector.tensor_copy(out=enc_natb[:], in_=enc_nat[:])
    for b in range(BL):
        for co in range(E // P):
            pst = ps_pro.tile([P, P], bf16, tag="pro", name=f"tp_{b}_{co}")
            nc.tensor.transpose(pst[:], enc_natb[:, b, ts(co, P)], ident)
            nc.vector.tensor_copy(out=seqT4[:, co, b, :], in_=pst[:])
    decT = gpool.tile([P, KO, BL], f32)
    for b in range(BL):
        nc.sync.dma_start(out=decT[:, :, b],
                          in_=ext["dec"][b].rearrange("(ko p) -> p ko", p=P))
    decTb = gpool.tile([P, KO, BL], bf16)
    nc.vector.tensor_copy(out=decTb[:], in_=decT[:])
    nc.vector.tensor_copy(out=seqT4[:, E // P:SO],
                          in_=decTb[:, :, :, None].to_broadcast((P, KO, BL, T)))

    def load_cast(dst3, src_ext, nck, width, tag, scale=None):
        src_c = src_ext.rearrange("(k p) m -> k p m", p=P)
        for k in range(nck):
            tmp = wtmp_pool.tile([P, width], f32, tag=tag, name=f"{tag}_{k}")
            nc.sync.dma_start(out=tmp[:], in_=src_c[k])
            if scale is None:
                nc.vector.tensor_copy(out=dst3[:, k], in_=tmp[:])
            else:
                nc.vector.tensor_scalar_mul(dst3[:, k], tmp[:], scale)

    load_cast(Wkb, ext["gk"], SO, 3 * H, "wtmp")
    load_cast(Rb, ext["gr"], KO, 3 * H, "wtmp", scale=SCALE_R)
    if has_b3:
        b3_brd = bass.AP(tensor=ext["b3"].tensor, offset=ext["b3"].offset,
                         ap=[[0, P]] + list(ext["b3"].ap))
        nc.sync.dma_start(out=b3bc[:], in_=b3_brd)

    if has_gb:
        gbT = gpool.tile([P, MO3, 2], f32)
        for i in range(2):
            nc.sync.dma_start(out=gbT[:, :, i],
                              in_=ext["gb"][i].rearrange("(mo p) -> p mo", p=P))
        xbias = gpool.tile([P, MO3], f32)
        nc.vector.tensor_copy(out=xbias[:], in_=gbT[:, :, 0])
        nc.vector.tensor_add(out=xbias[:, 0:8], in0=xbias[:, 0:8], in1=gbT[:, 0:8, 1])
        nc.vector.tensor_scalar_mul(xbias[:, 0:8], xbias[:, 0:8], SCALE_R)
        brecH_h = gpool.tile([P, KO, BL], f32)   # bias[1] h-part, halved
        nc.vector.tensor_copy(out=brecH_h[:],
                              in_=gbT[:, 8:12, 1:2].to_broadcast((P, KO, BL)))
        nc.vector.tensor_scalar_mul(brecH_h[:], brecH_h[:], 0.5)

    # w1/w2/biases loaded up front (DVE casts land in the first steps' gaps)
    load_cast(w1b, ext["w1"], KO, H, "wtmp")
    load_cast(w2b, ext["w2"], KO, H, "wtmp")
    nc.sync.dma_start(out=b1T[:], in_=ext["b1"].rearrange("(mo p) -> p mo", p=P))
    nc.sync.dma_start(out=b2T[:], in_=ext["b2"].rearrange("(mo p) -> p mo", p=P))
    nc.vector.tensor_scalar_mul(b2T[:], b2T[:], SCALE_H2)

    # ---------------- x-projection ----------------
    # z/r blocks (m 0-7) stored x SCALE_R; h block (m 8-11) unscaled.
    XC = 4
    XS = T // XC
    seq_bt = seqT.rearrange("p so (b t) -> p so b t", b=BL)
    xp_bt = xprojT.rearrange("p m (b t) -> p m b t", b=BL)

    def emit_xproj_piece(c, m0, nm):
        for m in range(m0, m0 + nm):
            psx = ps_pro.tile([P, BL * XS], f32, tag="pro", name=f"xp_{c}_{m}")
            for k in range(SO):
                nc.tensor.matmul(psx[:], lhsT=Wkb[:, k, ts(m, P)],
                                 rhs=seq_bt[:, k, :, ds(XS * c, XS)],
                                 start=(k == 0), stop=(k == SO - 1))
            dst = xp_bt[:, m, :, ds(XS * c, XS)]
            sc = SCALE_R if m < 8 else 1.0
            if has_gb:
                nc.scalar.activation(out=dst, in_=psx[:], func=Ident,
                                     bias=xbias[:, m:m + 1], scale=sc)
            else:
                nc.scalar.activation(out=dst, in_=psx[:], func=Copy, scale=sc)

    emit_xproj_piece(0, 0, MO3)   # chunk 0 fully up front

    # ---------------- deferred-work machinery ----------------
    crit_q = deque()   # (ready_step, closure, cost)
    bulk_q = deque()

    def pump(t, budget=1.6):
        for q in (crit_q, bulk_q):
            while q and budget > 0:
                ready, fn, cost = q[0]
                if ready > t:
                    break
                q.popleft()
                fn()
                budget -= cost

    def drain_all():
        for q in (crit_q, bulk_q):
            while q:
                _, fn, _ = q.popleft()
                fn()

    # w3 load+cast in 8 half-row chunks -> 16 cast pieces, spread early
    w3_c = ext["w3"].rearrange("(k p) m -> k p m", p=P)

    def push_w3_loads():
        for k in range(KO):
            for hlf in range(2):
                def mk(k=k, hlf=hlf):
                    def go():
                        tmp = wtmp_pool.tile([P, VS // 2], f32, tag="w3tmp",
                                             name=f"w3t_{k}_{hlf}")
                        nc.sync.dma_start(out=tmp[:],
                                          in_=w3_c[k][:, ds(hlf * VS // 2, VS // 2)])
                        nc.vector.tensor_scalar_mul(
                            w3b[:, k, ds(hlf * VS // 2, VS // 2)], tmp[:], SCALE_W3)
                    return go
                crit_q.append((2 + 3 * (2 * k + hlf), mk(), 1.0))

    push_w3_loads()

    # ---------------- vocab tile + softmax round machinery ----------------
    hs4 = hseqT.rearrange("p ko (b t) -> p ko b t", b=BL)
    h284 = h28T.rearrange("p ko (b t) -> p ko b t", b=BL)
    out3 = ext["out"].rearrange("(e t) v -> e t v", t=T)

    rounds = []               # round objects created at queue time
    tiles_queued = [0]

    def tile_meta(c, i):
        ch = CHS[c]
        return dict(c=c, i=i, ch=ch, s=CH_STARTS[c])

    def emit_tile_slots(meta, ready):
        """5 slot-closures per tile: slot k emits exp(k-1) then MM-group(k)."""
        c, i, ch, s = meta["c"], meta["i"], meta["ch"], meta["s"]
        lhs_base = h2g8[c]
        pv_tiles = {}
        expb = exp_pool.tile([P, NJ, VC], bf16, tag="expb", name=f"expb_{c}_{i}")
        if tiles_queued[0] % ROUND == 0:
            rounds.append(dict(
                rnd=len(rounds),
                sums=sc_pool.tile([P, ROUND * 4], f32, tag="sums",
                                  name=f"sums_{len(rounds)}"),
                exps=[], metas=[], done=[0]))
        tiles_queued[0] += 1
        rd_obj = rounds[-1]
        sums_t = rd_obj["sums"]
        ti = len(rd_obj["exps"])
        rd_obj["exps"].append(expb)
        rd_obj["metas"].append(meta)

        def mm_group(g):
            pv = ps_voc.tile([P, 2, 512], f32, tag="pv", name=f"pv_{c}_{i}_{g}")
            pv_tiles[g] = pv
            for jx in range(2):
                jj = 2 * g + jx
                for k2 in range(KO // 2):
                    nc.tensor.matmul(pv[:, jx, 0:VC],
                                     lhsT=lhs_base[:, 2 * k2:2 * k2 + 2, ds(P * i, P)],
                                     rhs=w3b[:, 2 * k2:2 * k2 + 2, ds(VC * jj, VC)],
                                     start=(k2 == 0), stop=(k2 == KO // 2 - 1),
                                     perf_mode=DR)

        def exp_group(g):
            pv = pv_tiles.pop(g)
            if has_b3:
                b3v = b3bc[:, ds(VC * 2 * g, VC * 2)].rearrange(
                    "p (j v) -> p j v", j=2)
                nc.vector.scalar_tensor_tensor(
                    out=pv[:, :, 0:VC], in0=pv[:, :, 0:VC], scalar=INV_LOGIT,
                    in1=b3v, op0=Mult, op1=Add)
                nc.scalar.activation(out=expb[:, ds(2 * g, 2), :],
                                     in_=pv[:, :, 0:VC], func=Exp,
                                     accum_out=sums_t[:, 4 * ti + g:4 * ti + g + 1])
            else:
                nc.scalar.activation(out=expb[:, ds(2 * g, 2), :],
                                     in_=pv[:, :, 0:VC], func=Exp, scale=INV_LOGIT,
                                     accum_out=sums_t[:, 4 * ti + g:4 * ti + g + 1])

        def slot(k):
            def go():
                if k > 0:
                    exp_group(k - 1)
                if k < 4:
                    mm_group(k)
                if k == 4:
                    import os as _os4
                    if _os4.environ.get("DBG_EXP") and c == 0 and i == 0:
                        nc.sync.dma_start(out=ext["dbg5"], in_=expb[:])
                    rd_obj["done"][0] += 1
                    if rd_obj["done"][0] == ROUND:
                        finish_round(rd_obj)
            return go

        for k in range(5):
            bulk_q.append((ready + k, slot(k), 1.0 if k < 4 else 1.2))

    def finish_round(st):
        rnd = st["rnd"]
        sums_t = st["sums"]
        sv = sums_t.rearrange("p (i g) -> p i g", g=4)
        ssum = sc_pool.tile([P, ROUND], f32, tag="ssum", name=f"ssum_{rnd}")
        tmp2 = sc_pool.tile([P, ROUND], f32, tag="ssum2", name=f"ssum2_{rnd}")
        nc.vector.tensor_add(out=ssum[:], in0=sv[:, :, 0], in1=sv[:, :, 1])
        nc.vector.tensor_add(out=tmp2[:], in0=sv[:, :, 2], in1=sv[:, :, 3])
        nc.vector.tensor_add(out=ssum[:], in0=ssum[:], in1=tmp2[:])
        nc.gpsimd.dma_start(out=sums_in[rnd].rearrange("(i p) -> p i", p=P),
                            in_=ssum[:])
        import os as _os2
        if _os2.environ.get("DBG_SUMS"):
            nc.sync.dma_start(out=ext["dbg3"][rnd, 0], in_=ssum[:])
        nc.gpsimd.collective_compute(
            "AllReduce", mybir.AluOpType.add,
            ins=[sums_in[rnd].opt()], outs=[sums_out[rnd].opt()],
            replica_groups=RG)
        # queue the scale-out of the PREVIOUS round now that its reduce has
        # had a full round of slack
        if rnd >= 1:
            push_round_scales(rounds[rnd - 1])

    def push_round_scales(st):
        # MUST be emitted before the round that reuses this round's expb
        # pool slots (bufs = 2 rounds), so these closures go to the FRONT
        # of the bulk queue — Tile dependencies follow emission order.
        rnd = st["rnd"]
        rcp = sc_pool.tile([P, ROUND], f32, tag="rcp", name=f"rcp_{rnd}")

        def rd():
            nc.sync.dma_start(out=rcp[:],
                              in_=sums_out[rnd].rearrange("(i p) -> p i", p=P))
            import os as _os3
            if _os3.environ.get("DBG_SUMS"):
                nc.sync.dma_start(out=ext["dbg3"][rnd, 1], in_=rcp[:])
            nc.vector.reciprocal(out=rcp[:], in_=rcp[:])

        items = [(0, rd, 0.4)]
        for ti, (expb, meta) in enumerate(zip(st["exps"], st["metas"])):
            ob = out_pool.tile([P, NJ, VC], bf16, tag="ob",
                               name=f"ob_{meta['c']}_{meta['i']}")

            def mk_scale(expb=expb, ob=ob, ti=ti, half=0):
                def go():
                    src = expb[:, ds(half * 4, 4), :]
                    dst = ob[:, ds(half * 4, 4), :]
                    nc.vector.tensor_scalar_mul(dst, src, rcp[:, ti:ti + 1])
                return go

            def mk_dma(ob=ob, meta=meta):
                def go():
                    c, i, ch, s = meta["c"], meta["i"], meta["ch"], meta["s"]
                    ne = P // ch           # examples per tile
                    obf = ob.rearrange("p j v -> p (j v)")
                    for e in range(ne):
                        # SBUF partition dim must stay whole per DMA
                        nc.sync.dma_start(
                            out=out3[i * ne + e, ds(s, ch), :],
                            in_=obf[ds(e * ch, ch), :])
                return go

            for half in range(2):
                items.append((0, mk_scale(half=half), 0.6))
            items.append((0, mk_dma(), 0.1))
        for it in reversed(items):
            bulk_q.appendleft(it)

    # ---------------- MLP + gather per chunk ----------------
    def emit_mlp_piece(c, h1c, layer, m0):
        s, ch = CH_STARTS[c], CHS[c]
        for m in range(m0, m0 + 2):
            psm = ps_pro.tile([P, BL * ch], f32, tag="pro", name=f"ml{layer}_{c}_{m}")
            for k in range(KO):
                rhs = (hs4[:, k, :, ds(s, ch)] if layer == 1 else h1c[:, k])
                nc.tensor.matmul(psm[:], lhsT=(w1b if layer == 1 else w2b)[:, k, ts(m, P)],
                                 rhs=rhs, start=(k == 0), stop=(k == KO - 1))
            if layer == 1:
                nc.scalar.activation(out=h1c[:, m], in_=psm[:],
                                     func=Relu, bias=b1T[:, m:m + 1])
            else:
                nc.scalar.activation(
                    out=h284[:, m, :, ds(s, ch)],
                    in_=psm.rearrange("p (b t) -> p b t", b=BL),
                    func=Relu, bias=b2T[:, m:m + 1], scale=SCALE_H2)

    def emit_gather(c):
        s, ch = CH_STARTS[c], CHS[c]
        bnc = h2_bounce[c].rearrange("(ko p) (b tt) -> ko p b tt", p=P, b=BL)
        for ko in range(KO):
            nc.gpsimd.dma_start(out=bnc[ko], in_=h284[:, ko, :, ds(s, ch)])
        nc.gpsimd.collective_compute(
            "AllGather", mybir.AluOpType.bypass,
            ins=[h2_bounce[c].opt()], outs=[h2_gath[c].opt()],
            replica_groups=RG)
        src = h2_gath[c].rearrange("(r ko p) t -> ko p r t", p=P, ko=KO)
        dstv = h2g8[c].rearrange("p ko (r tt) -> p ko r tt", r=NCORES)
        for ko in range(KO):
            nc.sync.dma_start(out=dstv[:, ko], in_=src[ko])

    def chunk_tail(c, t_now):
        """Emit after the last GRU step of chunk c: MLP, bounce, AG, tiles."""
        h1c = h1_pool.tile([P, KO, BL * CHS[c]], bf16, tag="h1c",
                           name=f"h1c_{c}")
        crit_q.append((t_now, lambda c=c, h=h1c: emit_mlp_piece(c, h, 1, 0), 1.0))
        crit_q.append((t_now + 1, lambda c=c, h=h1c: emit_mlp_piece(c, h, 1, 2), 1.0))
        crit_q.append((t_now + 2, lambda c=c, h=h1c: emit_mlp_piece(c, h, 2, 0), 1.0))
        crit_q.append((t_now + 3, lambda c=c, h=h1c: emit_mlp_piece(c, h, 2, 2), 1.0))
        crit_q.append((t_now + 4, lambda c=c: emit_gather(c), 0.5))
        ntiles_c = CHS[c] * BL * NCORES // P
        for i in range(ntiles_c):
            meta = tile_meta(c, i)
            emit_tile_slots(meta, ready=t_now + AG_MARGIN + i)

    # ---------------- GRU scan ----------------
    xp4 = xp_bt
    HALF_INV_R = INV_R / 2.0
    U_SCALE = 1.0 / (2.0 * SCALE_R)

    # t = 0 (h == 0)
    zr0 = gt_pool.tile([P, 8, BL], f32, tag="tnh")
    nc.scalar.activation(out=zr0[:], in_=xp4[:, 0:8, :, 0], func=Tanh,
                         scale=HALF_INV_R)
    hh0 = gt_pool.tile([P, KO, BL], f32, tag="hh")
    if has_gb:
        # hh = relu(xh + r*brec) ; r = (r'+1)/2
        nc.vector.scalar_tensor_tensor(out=hh0[:], in0=zr0[:, 4:8], scalar=1.0,
                                       in1=brecH_h[:], op0=Add, op1=Mult)
        nc.vector.tensor_add(out=hh0[:], in0=hh0[:], in1=xp4[:, 8:12, :, 0])
        nc.vector.tensor_scalar_max(hh0[:], hh0[:], 0.0)
    else:
        nc.vector.tensor_scalar_max(hh0[:], xp4[:, 8:12, :, 0], 0.0)
    f0 = gt_pool.tile([P, KO, BL], f32, tag="f")
    nc.vector.scalar_tensor_tensor(out=f0[:], in0=zr0[:, 0:4], scalar=1.0,
                                   in1=hh0[:], op0=Sub, op1=Mult)
    nc.vector.tensor_scalar_mul(hs4[:, :, :, 0], f0[:], -0.5)

    chunk_of_step = {}
    for c in range(NCH):
        chunk_of_step[CH_STARTS[c] + CHS[c] - 1] = c

    for t in range(1, T):
        if t % XS == XS - 10 and t // XS + 1 < XC:
            c = t // XS + 1
            emit_xproj_piece(c, 0, 6)
        if t % XS == XS - 5 and t // XS + 1 < XC:
            c = t // XS + 1
            emit_xproj_piece(c, 6, 6)

        # PE: one [P,32] accumulation group: ident inject (r+z), then r
        # chunks (cols 0:16), then z chunks (16:32).  tanh(r) reads the r
        # columns as soon as their last matmul lands (subtile deps) and
        # overlaps the z/h matmuls.
        zr_ps = ps_gru.tile([P, 8 * BL], f32, tag="zr_ps", name=f"zrp_{t}")
        h_ps = ps_gru.tile([P, KO * BL], f32, tag="h_ps", name=f"hp_{t}")
        xp_rz = xp4[:, 4:8, :, t], xp4[:, 0:4, :, t]
        nc.tensor.matmul(zr_ps[:, 0:16], lhsT=ident, rhs=xp_rz[0],
                         start=True, stop=False, skip_group_check=True)
        nc.tensor.matmul(zr_ps[:, 16:32], lhsT=ident, rhs=xp_rz[1],
                         start=False, stop=False, skip_group_check=True)
        for m in range(4):
            for ko in range(KO):
                nc.tensor.matmul(zr_ps[:, ds(BL * m, BL)],
                                 lhsT=Rb[:, ko, ts(4 + m, P)],
                                 rhs=hs4[:, ko, :, t - 1],
                                 start=False, stop=False, skip_group_check=True)
        for m in range(4):
            for ko in range(KO):
                nc.tensor.matmul(zr_ps[:, ds(16 + BL * m, BL)],
                                 lhsT=Rb[:, ko, ts(m, P)],
                                 rhs=hs4[:, ko, :, t - 1],
                                 start=False, stop=(ko == KO - 1) and (m == 3),
                                 skip_group_check=True)
        for m in range(4):
            for ko in range(KO):
                nc.tensor.matmul(h_ps[:, ds(BL * m, BL)],
                                 lhsT=Rb[:, ko, ts(8 + m, P)],
                                 rhs=hs4[:, ko, :, t - 1],
                                 start=(ko == 0), stop=(ko == KO - 1))
        # ACT: tanh(r) early (overlaps z/h matmuls), tanh(z) after
        rp_ = gt_pool.tile([P, 4, BL], f32, tag="tnh", name=f"rp_{t}")
        nc.scalar.activation(out=rp_[:],
                             in_=zr_ps[:, 0:16].rearrange("p (m b) -> p m b", b=BL),
                             func=Tanh, scale=HALF_INV_R)
        zp_ = gt_pool.tile([P, 4, BL], f32, tag="tnhz", name=f"zp_{t}")
        nc.scalar.activation(out=zp_[:],
                             in_=zr_ps[:, 16:32].rearrange("p (m b) -> p m b", b=BL),
                             func=Tanh, scale=HALF_INV_R)
        # DVE 5-op STT chain
        hp4 = h_ps.rearrange("p (m b) -> p m b", b=BL)
        u = gt_pool.tile([P, KO, BL], f32, tag="u", name=f"u_{t}")
        nc.vector.scalar_tensor_tensor(out=u[:], in0=rp_[:], scalar=1.0,
                                       in1=hp4, op0=Add, op1=Mult)
        w = gt_pool.tile([P, KO, BL], f32, tag="w", name=f"w_{t}")
        if has_gb:
            ub = gt_pool.tile([P, KO, BL], f32, tag="ub", name=f"ub_{t}")
            nc.vector.scalar_tensor_tensor(out=ub[:], in0=rp_[:], scalar=1.0,
                                           in1=brecH_h[:], op0=Add, op1=Mult)
            nc.vector.scalar_tensor_tensor(out=w[:], in0=u[:], scalar=U_SCALE,
                                           in1=ub[:], op0=Mult, op1=Add)
            nc.vector.tensor_add(out=w[:], in0=w[:], in1=xp4[:, 8:12, :, t])
        else:
            nc.vector.scalar_tensor_tensor(out=w[:], in0=u[:], scalar=U_SCALE,
                                           in1=xp4[:, 8:12, :, t], op0=Mult, op1=Add)
        dd = gt_pool.tile([P, KO, BL], f32, tag="d", name=f"d_{t}")
        nc.vector.scalar_tensor_tensor(out=dd[:], in0=w[:], scalar=0.0,
                                       in1=hs4[:, :, :, t - 1], op0=Max, op1=Sub)
        ff = gt_pool.tile([P, KO, BL], f32, tag="f", name=f"f_{t}")
        nc.vector.scalar_tensor_tensor(out=ff[:], in0=zp_[:], scalar=1.0,
                                       in1=dd[:], op0=Sub, op1=Mult)
        nc.vector.scalar_tensor_tensor(out=hs4[:, :, :, t], in0=ff[:], scalar=-0.5,
                                       in1=hs4[:, :, :, t - 1], op0=Mult, op1=Add)

        if t in chunk_of_step:
            chunk_tail(chunk_of_step[t], t + 1)
        pump(t)

    # ---------------- drain ----------------
    import os
    if os.environ.get("DBG_W3"):
        nc.sync.dma_start(out=ext["dbg4"], in_=w3b[:])
    if os.environ.get("DBG_HSEQ"):
        dbgf = gpool.tile([P, KO, NTOK], f32)
        nc.vector.tensor_copy(out=dbgf[:], in_=hseqT[:])
        nc.sync.dma_start(out=ext["dbg"], in_=dbgf[:])
    drain_all()
    if os.environ.get("DBG_H2G"):
        off = 0
        for c in range(NCH):
            n = NCORES * CHS[c] * BL
            nc.sync.dma_start(out=ext["dbg2"][:, :, ds(off, n)], in_=h2g8[c][:])
            off += n
    if rounds:
        push_round_scales(rounds[-1])
        drain_all()

    stack.close()


_BUILD_CACHE = {}


def _get_nc(has_b3: bool, has_gb: bool):
    key = (has_b3, has_gb)
    if key not in _BUILD_CACHE:
        _BUILD_CACHE[key] = _build(has_b3, has_gb)
    return _BUILD_CACHE[key]


def _make_in_maps(inputs):
    arrs = {k: np.ascontiguousarray(np.asarray(v, dtype=np.float32))
            for k, v in inputs.items()}
    in_maps = []
    for c in range(NCORES):
        in_maps.append({
            "encoder_input": arrs["encoder_input"][BL * c:BL * (c + 1)],
            "decoder_input": arrs["decoder_input"][BL * c:BL * (c + 1)],
            "gru_kernel": arrs["gru_kernel"],
            "gru_rec_kernel": arrs["gru_rec_kernel"],
            "gru_bias": arrs["gru_bias"],
            "w1": arrs["w1"], "b1": arrs["b1"],
            "w2": arrs["w2"], "b2": arrs["b2"],
            "w3": np.ascontiguousarray(arrs["w3"][:, VS * c:VS * (c + 1)]),
            "b3": np.ascontiguousarray(arrs["b3"][VS * c:VS * (c + 1)]),
        })
    flags = (bool(np.any(arrs["b3"])), bool(np.any(arrs["gru_bias"])))
    return in_maps, flags


def kernel(**inputs):
    global LAST_RESULT
    in_maps, (has_b3, has_gb) = _make_in_maps(inputs)
    nc = _get_nc(has_b3, has_gb)
    res = run_bass_kernel_spmd(nc, in_maps, core_ids=list(range(NCORES)),
                               trace=TRACE, **TRACE_KWARGS)
    LAST_RESULT = res
    full = np.empty((B, T, V), np.float32)
    for c in range(NCORES):
        full[:, :, VS * c:VS * (c + 1)] = \
            res.results[c]["out"].astype(np.float32).reshape(B, T, VS)
    return full


# revision 27
# speedup vs baseline: 1.1350x; 1.1350x over previous
"""Trainium2 Bass kernel for nn_Decoder (GRU decoder + MLP + vocab softmax).

Sharding (8 NeuronCores):
  - GRU + 2-layer MLP: data-parallel over batch (4 examples/core).
    Local tokens are b-major (col = b*128 + t) so the global token index
    G = 512*rank + b*128 + t equals example*128 + t, matching output rows.
  - h2^T (fp8 x16) all-gathered across cores in 6 step-chunks DURING the
    GRU scan; vocab tiles cover (step-chunk x ranks) so the [512,32000]
    column-parallel projection + softmax is interleaved into the GRU's
    engine gaps instead of running as a separate phase.

Precision: recurrent kernel fp8e4 (x32), vocab matmul fp8xfp8 DoubleRow
(h2 x16, w3 x64), else bf16 operands / fp32 PSUM.  Output bf16, upcast on
host.  ~1.3e-2 rel err vs fp32 reference (gate 2e-2).

GRU gates use Tanh (same ScalarE table set as Exp, so interleaving vocab
exp causes no ACT table reloads): sigmoid(x) = (tanh(x/2)+1)/2, folded
into a 5-op scalar_tensor_tensor chain on VectorE:
    u  = (r' + 1) * h_ps            # = 64 * r * rh
    w  = (u / 64) + xh
    d  = max(w, 0) - h_prev         # = hh - h_prev
    f  = (z' - 1) * d               # = (1-z')(h_prev-hh)
    hs = (f * -0.5) + h_prev        # = z*h_prev + (1-z)*hh
"""

from collections import deque

import numpy as np

import concourse.bass as bass
import concourse.tile as tile
from concourse import bacc, mybir
from concourse.bass import ds, ts
from concourse.bass_utils import run_bass_kernel_spmd
from concourse.masks import make_identity

P = 128
NCORES = 8
B, T, E, H, V = 32, 128, 256, 512, 32000
BL = B // NCORES            # 4 examples per core
NTOK = BL * T               # 512 local tokens
G = B * T                   # 4096 global tokens
VS = V // NCORES            # 4000 vocab cols per core
KO = H // P                 # 4 hidden chunks
MO3 = 3 * H // P            # 12 gate chunks
SO = (E + H) // P           # 6 input chunks
NJ = 8                      # vocab col chunks per token tile (8 x 500)
VC = VS // NJ               # 500

CHS = [16, 16, 32, 32, 16, 16]          # GRU steps per gather chunk
CH_STARTS = [sum(CHS[:i]) for i in range(len(CHS))]
NCH = len(CHS)
ROUND = 4                               # tiles per softmax all-reduce round
NTILES = G // P                         # 32
NROUNDS = NTILES // ROUND               # 8
import os as _os
AG_MARGIN = int(_os.environ.get("AG_MARGIN", "10"))     # steps before a chunk's tiles unlock

SCALE_R = 32.0
SCALE_H2 = 16.0
SCALE_W3 = 64.0
INV_R = 1.0 / SCALE_R
INV_LOGIT = 1.0 / (SCALE_H2 * SCALE_W3)

f32 = mybir.dt.float32
bf16 = mybir.dt.bfloat16
fp8 = mybir.dt.float8e4

TRACE = False
TRACE_KWARGS = {}
LAST_RESULT = None

RG = [list(range(NCORES))]
DR = mybir.MatmulPerfMode.DoubleRow

Copy = mybir.ActivationFunctionType.Copy
Ident = mybir.ActivationFunctionType.Identity
Tanh = mybir.ActivationFunctionType.Tanh
Relu = mybir.ActivationFunctionType.Relu
Exp = mybir.ActivationFunctionType.Exp
Add = mybir.AluOpType.add
Sub = mybir.AluOpType.subtract
Mult = mybir.AluOpType.mult
Max = mybir.AluOpType.max


def _build(has_b3: bool, has_gb: bool):
    nc = bacc.Bacc("TRN2", target_bir_lowering=False, debug=False,
                   num_devices=NCORES)

    ext = {}
    ext["enc"] = nc.dram_tensor("encoder_input", [BL, T, E], f32, kind="ExternalInput").ap()
    ext["dec"] = nc.dram_tensor("decoder_input", [BL, H], f32, kind="ExternalInput").ap()
    ext["gk"] = nc.dram_tensor("gru_kernel", [E + H, 3 * H], f32, kind="ExternalInput").ap()
    ext["gr"] = nc.dram_tensor("gru_rec_kernel", [H, 3 * H], f32, kind="ExternalInput").ap()
    ext["gb"] = nc.dram_tensor("gru_bias", [2, 3 * H], f32, kind="ExternalInput").ap()
    ext["w1"] = nc.dram_tensor("w1", [H, H], f32, kind="ExternalInput").ap()
    ext["b1"] = nc.dram_tensor("b1", [H], f32, kind="ExternalInput").ap()
    ext["w2"] = nc.dram_tensor("w2", [H, H], f32, kind="ExternalInput").ap()
    ext["b2"] = nc.dram_tensor("b2", [H], f32, kind="ExternalInput").ap()
    ext["w3"] = nc.dram_tensor("w3", [H, VS], f32, kind="ExternalInput").ap()
    ext["b3"] = nc.dram_tensor("b3", [VS], f32, kind="ExternalInput").ap()
    ext["out"] = nc.dram_tensor("out", [G, VS], bf16, kind="ExternalOutput").ap()
    import os
    if os.environ.get("DBG_HSEQ"):
        ext["dbg"] = nc.dram_tensor("dbg", [P, KO, NTOK], f32, kind="ExternalOutput").ap()
    if os.environ.get("DBG_H2G"):
        ext["dbg2"] = nc.dram_tensor("dbg2", [P, KO, G], fp8, kind="ExternalOutput").ap()
    if os.environ.get("DBG_SUMS"):
        ext["dbg3"] = nc.dram_tensor("dbg3", [NROUNDS, 2, P, ROUND], f32, kind="ExternalOutput").ap()
    if os.environ.get("DBG_W3"):
        ext["dbg4"] = nc.dram_tensor("dbg4", [P, KO, VS], fp8, kind="ExternalOutput").ap()
    if os.environ.get("DBG_EXP"):
        ext["dbg5"] = nc.dram_tensor("dbg5", [P, NJ, VC], bf16, kind="ExternalOutput").ap()

    with tile.TileContext(nc) as tc:
        with tc.tile_pool(name="dram", bufs=1, space="DRAM") as dram_pool:
            h2_bounce = [dram_pool.tile([H, CHS[c] * BL], fp8, name=f"h2b_{c}")
                         for c in range(NCH)]
            h2_gath = [dram_pool.tile([NCORES * H, CHS[c] * BL], fp8,
                                      addr_space="Shared", name=f"h2g_{c}")
                       for c in range(NCH)]
            sums_in = [dram_pool.tile([P * ROUND], f32, name=f"sums_in_{r}")
                       for r in range(NROUNDS)]
            sums_out = [dram_pool.tile([P * ROUND], f32, addr_space="Shared",
                                       name=f"sums_out_{r}")
                        for r in range(NROUNDS)]
            _body(nc, tc, has_b3, has_gb, ext,
                  h2_bounce, h2_gath, sums_in, sums_out)
    nc.finalize()
    return nc


def _body(nc, tc, has_b3, has_gb, ext, h2_bounce, h2_gath, sums_in, sums_out):
    from contextlib import ExitStack

    stack = ExitStack()
    gpool = stack.enter_context(tc.tile_pool(name="gpool", bufs=1))
    wtmp_pool = stack.enter_context(tc.tile_pool(name="wtmp", bufs=2))
    gt_pool = stack.enter_context(tc.tile_pool(name="gt", bufs=3))
    h1_pool = stack.enter_context(tc.tile_pool(name="h1p", bufs=2))
    exp_pool = stack.enter_context(tc.tile_pool(name="exp", bufs=8))
    out_pool = stack.enter_context(tc.tile_pool(name="outp", bufs=2))
    sc_pool = stack.enter_context(tc.tile_pool(name="scp", bufs=3))
    ps_gru = stack.enter_context(tc.tile_pool(name="ps_gru", bufs=1, space="PSUM"))
    ps_pro = stack.enter_context(tc.tile_pool(name="ps_pro", bufs=2, space="PSUM"))
    ps_voc = stack.enter_context(tc.tile_pool(name="ps_voc", bufs=2, space="PSUM"))

    w3b = gpool.tile([P, KO, VS], fp8)
    b3bc = gpool.tile([P, VS], f32, name="b3bc") if has_b3 else None
    w1b = gpool.tile([P, KO, H], bf16)
    w2b = gpool.tile([P, KO, H], bf16)
    b1T = gpool.tile([P, KO], f32)
    b2T = gpool.tile([P, KO], f32)
    Rb = gpool.tile([P, KO, 3 * H], fp8)
    Wkb = gpool.tile([P, SO, 3 * H], bf16)
    seqT = gpool.tile([P, SO, NTOK], bf16)
    xprojT = gpool.tile([P, MO3, NTOK], bf16)
    hseqT = gpool.tile([P, KO, NTOK], bf16)
    h28T = gpool.tile([P, KO, NTOK], fp8)
    h2g8 = [gpool.tile([P, KO, NCORES * CHS[c] * BL], fp8, name=f"h2g8_{c}")
            for c in range(NCH)]

    # ---------------- initial loads ----------------
    seqT4 = seqT.rearrange("p so (b t) -> p so b t", b=BL)
    ident = gpool.tile([P, P], bf16)
    make_identity(nc, ident)
    enc_nat = wtmp_pool.tile([P, BL, E], f32, tag="wtmp")
    nc.sync.dma_start(out=enc_nat[:], in_=ext["enc"].rearrange("b t c -> t b c"))
    enc_natb = wtmp_pool.tile([P, BL, E], bf16, tag="encb", bufs=1)
    nc.vector.tensor_copy(out=enc_natb[:], in_=enc_nat[:])
    for b in range(BL):
        for co in range(E // P):
            pst = ps_pro.tile([P, P], bf16, tag="pro", name=f"tp_{b}_{co}")
            nc.tensor.transpose(pst[:], enc_natb[:, b, ts(co, P)], ident)
            nc.vector.tensor_copy(out=seqT4[:, co, b, :], in_=pst[:])
    decT = gpool.tile([P, KO, BL], f32)
    for b in range(BL):
        nc.sync.dma_start(out=decT[:, :, b],
                          in_=ext["dec"][b].rearrange("(ko p) -> p ko", p=P))
    decTb = gpool.tile([P, KO, BL], bf16)
    nc.vector.tensor_copy(out=decTb[:], in_=decT[:])
    nc.vector.tensor_copy(out=seqT4[:, E // P:SO],
                          in_=decTb[:, :, :, None].to_broadcast((P, KO, BL, T)))

    def load_cast(dst3, src_ext, nck, width, tag, scale=None):
        src_c = src_ext.rearrange("(k p) m -> k p m", p=P)
        for k in range(nck):
            tmp = wtmp_pool.tile([P, width], f32, tag=tag, name=f"{tag}_{k}")
            nc.sync.dma_start(out=tmp[:], in_=src_c[k])
            if scale is None:
                nc.vector.tensor_copy(out=dst3[:, k], in_=tmp[:])
            else:
                nc.vector.tensor_scalar_mul(dst3[:, k], tmp[:], scale)

    load_cast(Wkb, ext["gk"], SO, 3 * H, "wtmp")
    load_cast(Rb, ext["gr"], KO, 3 * H, "wtmp", scale=SCALE_R)
    if has_b3:
        b3_brd = bass.AP(tensor=ext["b3"].tensor, offset=ext["b3"].offset,
                         ap=[[0, P]] + list(ext["b3"].ap))
        nc.sync.dma_start(out=b3bc[:], in_=b3_brd)

    if has_gb:
        gbT = gpool.tile([P, MO3, 2], f32)
        for i in range(2):
            nc.sync.dma_start(out=gbT[:, :, i],
                              in_=ext["gb"][i].rearrange("(mo p) -> p mo", p=P))
        xbias = gpool.tile([P, MO3], f32)
        nc.vector.tensor_copy(out=xbias[:], in_=gbT[:, :, 0])
        nc.vector.tensor_add(out=xbias[:, 0:8], in0=xbias[:, 0:8], in1=gbT[:, 0:8, 1])
        nc.vector.tensor_scalar_mul(xbias[:, 0:8], xbias[:, 0:8], SCALE_R)
        brecH_h = gpool.tile([P, KO, BL], f32)   # bias[1] h-part, halved
        nc.vector.tensor_copy(out=brecH_h[:],
                              in_=gbT[:, 8:12, 1:2].to_broadcast((P, KO, BL)))
        nc.vector.tensor_scalar_mul(brecH_h[:], brecH_h[:], 0.5)

    # w1/w2/biases loaded up front (DVE casts land in the first steps' gaps)
    load_cast(w1b, ext["w1"], KO, H, "wtmp")
    load_cast(w2b, ext["w2"], KO, H, "wtmp")
    nc.sync.dma_start(out=b1T[:], in_=ext["b1"].rearrange("(mo p) -> p mo", p=P))
    nc.sync.dma_start(out=b2T[:], in_=ext["b2"].rearrange("(mo p) -> p mo", p=P))
    nc.vector.tensor_scalar_mul(b2T[:], b2T[:], SCALE_H2)

    # ---------------- x-projection ----------------
    # z/r blocks (m 0-7) stored x SCALE_R; h block (m 8-11) unscaled.
    XC = 4
    XS = T // XC
    seq_bt = seqT.rearrange("p so (b t) -> p so b t", b=BL)
    xp_bt = xprojT.rearrange("p m (b t) -> p m b t", b=BL)

    def emit_xproj_piece(c, m0, nm):
        for m in range(m0, m0 + nm):
            psx = ps_pro.tile([P, BL * XS], f32, tag="pro", name=f"xp_{c}_{m}")
            for k in range(SO):
                nc.tensor.matmul(psx[:], lhsT=Wkb[:, k, ts(m, P)],
                                 rhs=seq_bt[:, k, :, ds(XS * c, XS)],
                                 start=(k == 0), stop=(k == SO - 1))
            dst = xp_bt[:, m, :, ds(XS * c, XS)]
            sc = SCALE_R if m < 8 else 1.0
            if has_gb:
                nc.scalar.activation(out=dst, in_=psx[:], func=Ident,
                                     bias=xbias[:, m:m + 1], scale=sc)
            else:
                nc.scalar.activation(out=dst, in_=psx[:], func=Copy, scale=sc)

    emit_xproj_piece(0, 0, MO3)   # chunk 0 fully up front

    # ---------------- deferred-work machinery ----------------
    crit_q = deque()   # (ready_step, closure, cost)
    bulk_q = deque()

    def pump(t, budget=float(__import__("os").environ.get("PUMP", "3.0"))):
        for q in (crit_q, bulk_q):
            while q and budget > 0:
                ready, fn, cost = q[0]
                if ready > t:
                    break
                q.popleft()
                fn()
                budget -= cost

    def drain_all():
        for q in (crit_q, bulk_q):
            while q:
                _, fn, _ = q.popleft()
                fn()

    # w3 load+cast in 8 half-row chunks -> 16 cast pieces, spread early
    w3_c = ext["w3"].rearrange("(k p) m -> k p m", p=P)

    def push_w3_loads():
        for k in range(KO):
            for hlf in range(2):
                def mk(k=k, hlf=hlf):
                    def go():
                        tmp = wtmp_pool.tile([P, VS // 2], f32, tag="w3tmp",
                                             name=f"w3t_{k}_{hlf}")
                        nc.sync.dma_start(out=tmp[:],
                                          in_=w3_c[k][:, ds(hlf * VS // 2, VS // 2)])
                        nc.vector.tensor_scalar_mul(
                            w3b[:, k, ds(hlf * VS // 2, VS // 2)], tmp[:], SCALE_W3)
                    return go
                crit_q.append((2 + 3 * (2 * k + hlf), mk(), 1.0))

    push_w3_loads()

    # ---------------- vocab tile + softmax round machinery ----------------
    hs4 = hseqT.rearrange("p ko (b t) -> p ko b t", b=BL)
    h284 = h28T.rearrange("p ko (b t) -> p ko b t", b=BL)
    out3 = ext["out"].rearrange("(e t) v -> e t v", t=T)

    rounds = []               # round objects created at queue time
    tiles_queued = [0]

    def tile_meta(c, i):
        ch = CHS[c]
        return dict(c=c, i=i, ch=ch, s=CH_STARTS[c])

    def emit_tile_slots(meta, ready):
        """5 slot-closures per tile: slot k emits exp(k-1) then MM-group(k)."""
        c, i, ch, s = meta["c"], meta["i"], meta["ch"], meta["s"]
        lhs_base = h2g8[c]
        pv_tiles = {}
        expb = exp_pool.tile([P, NJ, VC], bf16, tag="expb", name=f"expb_{c}_{i}")
        if tiles_queued[0] % ROUND == 0:
            rounds.append(dict(
                rnd=len(rounds),
                sums=sc_pool.tile([P, ROUND * 4], f32, tag="sums",
                                  name=f"sums_{len(rounds)}"),
                exps=[], metas=[], done=[0]))
        tiles_queued[0] += 1
        rd_obj = rounds[-1]
        sums_t = rd_obj["sums"]
        ti = len(rd_obj["exps"])
        rd_obj["exps"].append(expb)
        rd_obj["metas"].append(meta)

        def mm_group(g):
            pv = ps_voc.tile([P, 2, 512], f32, tag="pv", name=f"pv_{c}_{i}_{g}")
            pv_tiles[g] = pv
            for jx in range(2):
                jj = 2 * g + jx
                for k2 in range(KO // 2):
                    nc.tensor.matmul(pv[:, jx, 0:VC],
                                     lhsT=lhs_base[:, 2 * k2:2 * k2 + 2, ds(P * i, P)],
                                     rhs=w3b[:, 2 * k2:2 * k2 + 2, ds(VC * jj, VC)],
                                     start=(k2 == 0), stop=(k2 == KO // 2 - 1),
                                     perf_mode=DR)

        def exp_group(g):
            pv = pv_tiles.pop(g)
            if has_b3:
                b3v = b3bc[:, ds(VC * 2 * g, VC * 2)].rearrange(
                    "p (j v) -> p j v", j=2)
                nc.vector.scalar_tensor_tensor(
                    out=pv[:, :, 0:VC], in0=pv[:, :, 0:VC], scalar=INV_LOGIT,
                    in1=b3v, op0=Mult, op1=Add)
                nc.scalar.activation(out=expb[:, ds(2 * g, 2), :],
                                     in_=pv[:, :, 0:VC], func=Exp,
                                     accum_out=sums_t[:, 4 * ti + g:4 * ti + g + 1])
            else:
                nc.scalar.activation(out=expb[:, ds(2 * g, 2), :],
                                     in_=pv[:, :, 0:VC], func=Exp, scale=INV_LOGIT,
                                     accum_out=sums_t[:, 4 * ti + g:4 * ti + g + 1])

        def slot(k):
            def go():
                if k > 0:
                    exp_group(k - 1)
                if k < 4:
                    mm_group(k)
                if k == 4:
                    import os as _os4
                    if _os4.environ.get("DBG_EXP") and c == 0 and i == 0:
                        nc.sync.dma_start(out=ext["dbg5"], in_=expb[:])
                    rd_obj["done"][0] += 1
                    if rd_obj["done"][0] == ROUND:
                        finish_round(rd_obj)
            return go

        for k in range(5):
            bulk_q.append((ready + k // 2, slot(k), 1.0 if k < 4 else 1.2))

    def finish_round(st):
        rnd = st["rnd"]
        sums_t = st["sums"]
        sv = sums_t.rearrange("p (i g) -> p i g", g=4)
        ssum = sc_pool.tile([P, ROUND], f32, tag="ssum", name=f"ssum_{rnd}")
        tmp2 = sc_pool.tile([P, ROUND], f32, tag="ssum2", name=f"ssum2_{rnd}")
        nc.vector.tensor_add(out=ssum[:], in0=sv[:, :, 0], in1=sv[:, :, 1])
        nc.vector.tensor_add(out=tmp2[:], in0=sv[:, :, 2], in1=sv[:, :, 3])
        nc.vector.tensor_add(out=ssum[:], in0=ssum[:], in1=tmp2[:])
        nc.gpsimd.dma_start(out=sums_in[rnd].rearrange("(i p) -> p i", p=P),
                            in_=ssum[:])
        import os as _os2
        if _os2.environ.get("DBG_SUMS"):
            nc.sync.dma_start(out=ext["dbg3"][rnd, 0], in_=ssum[:])
        nc.gpsimd.collective_compute(
            "AllReduce", mybir.AluOpType.add,
            ins=[sums_in[rnd].opt()], outs=[sums_out[rnd].opt()],
            replica_groups=RG)
        # queue the scale-out of the PREVIOUS round now that its reduce has
        # had a full round of slack
        if rnd >= 1:
            push_round_scales(rounds[rnd - 1])

    def push_round_scales(st):
        # MUST be emitted before the round that reuses this round's expb
        # pool slots (bufs = 2 rounds), so these closures go to the FRONT
        # of the bulk queue — Tile dependencies follow emission order.
        rnd = st["rnd"]
        rcp = sc_pool.tile([P, ROUND], f32, tag="rcp", name=f"rcp_{rnd}")

        def rd():
            nc.sync.dma_start(out=rcp[:],
                              in_=sums_out[rnd].rearrange("(i p) -> p i", p=P))
            import os as _os3
            if _os3.environ.get("DBG_SUMS"):
                nc.sync.dma_start(out=ext["dbg3"][rnd, 1], in_=rcp[:])
            nc.vector.reciprocal(out=rcp[:], in_=rcp[:])

        items = [(0, rd, 0.4)]
        for ti, (expb, meta) in enumerate(zip(st["exps"], st["metas"])):
            ob = out_pool.tile([P, NJ, VC], bf16, tag="ob",
                               name=f"ob_{meta['c']}_{meta['i']}")

            def mk_scale(expb=expb, ob=ob, ti=ti, half=0):
                def go():
                    src = expb[:, ds(half * 4, 4), :]
                    dst = ob[:, ds(half * 4, 4), :]
                    nc.vector.tensor_scalar_mul(dst, src, rcp[:, ti:ti + 1])
                return go

            def mk_dma(ob=ob, meta=meta):
                def go():
                    c, i, ch, s = meta["c"], meta["i"], meta["ch"], meta["s"]
                    ne = P // ch           # examples per tile
                    obf = ob.rearrange("p j v -> p (j v)")
                    for e in range(ne):
                        # SBUF partition dim must stay whole per DMA
                        nc.sync.dma_start(
                            out=out3[i * ne + e, ds(s, ch), :],
                            in_=obf[ds(e * ch, ch), :])
                return go

            for half in range(2):
                items.append((0, mk_scale(half=half), 0.6))
            items.append((0, mk_dma(), 0.1))
        for it in reversed(items):
            bulk_q.appendleft(it)

    # ---------------- MLP + gather per chunk ----------------
    def emit_mlp_piece(c, h1c, layer, m0):
        s, ch = CH_STARTS[c], CHS[c]
        for m in range(m0, m0 + 2):
            psm = ps_pro.tile([P, BL * ch], f32, tag="pro", name=f"ml{layer}_{c}_{m}")
            for k in range(KO):
                rhs = (hs4[:, k, :, ds(s, ch)] if layer == 1 else h1c[:, k])
                nc.tensor.matmul(psm[:], lhsT=(w1b if layer == 1 else w2b)[:, k, ts(m, P)],
                                 rhs=rhs, start=(k == 0), stop=(k == KO - 1))
            if layer == 1:
                nc.scalar.activation(out=h1c[:, m], in_=psm[:],
                                     func=Relu, bias=b1T[:, m:m + 1])
            else:
                nc.scalar.activation(
                    out=h284[:, m, :, ds(s, ch)],
                    in_=psm.rearrange("p (b t) -> p b t", b=BL),
                    func=Relu, bias=b2T[:, m:m + 1], scale=SCALE_H2)

    def emit_gather(c):
        s, ch = CH_STARTS[c], CHS[c]
        bnc = h2_bounce[c].rearrange("(ko p) (b tt) -> ko p b tt", p=P, b=BL)
        for ko in range(KO):
            nc.gpsimd.dma_start(out=bnc[ko], in_=h284[:, ko, :, ds(s, ch)])
        nc.gpsimd.collective_compute(
            "AllGather", mybir.AluOpType.bypass,
            ins=[h2_bounce[c].opt()], outs=[h2_gath[c].opt()],
            replica_groups=RG)
        src = h2_gath[c].rearrange("(r ko p) t -> ko p r t", p=P, ko=KO)
        dstv = h2g8[c].rearrange("p ko (r tt) -> p ko r tt", r=NCORES)
        for ko in range(KO):
            nc.sync.dma_start(out=dstv[:, ko], in_=src[ko])

    def chunk_tail(c, t_now):
        """Emit after the last GRU step of chunk c: MLP, bounce, AG, tiles."""
        h1c = h1_pool.tile([P, KO, BL * CHS[c]], bf16, tag="h1c",
                           name=f"h1c_{c}")
        crit_q.append((t_now, lambda c=c, h=h1c: emit_mlp_piece(c, h, 1, 0), 1.0))
        crit_q.append((t_now + 1, lambda c=c, h=h1c: emit_mlp_piece(c, h, 1, 2), 1.0))
        crit_q.append((t_now + 2, lambda c=c, h=h1c: emit_mlp_piece(c, h, 2, 0), 1.0))
        crit_q.append((t_now + 3, lambda c=c, h=h1c: emit_mlp_piece(c, h, 2, 2), 1.0))
        crit_q.append((t_now + 4, lambda c=c: emit_gather(c), 0.5))
        ntiles_c = CHS[c] * BL * NCORES // P
        for i in range(ntiles_c):
            meta = tile_meta(c, i)
            emit_tile_slots(meta, ready=t_now + AG_MARGIN + i)

    # ---------------- GRU scan ----------------
    xp4 = xp_bt
    HALF_INV_R = INV_R / 2.0
    U_SCALE = 1.0 / (2.0 * SCALE_R)

    # t = 0 (h == 0)
    zr0 = gt_pool.tile([P, 8, BL], f32, tag="tnh")
    nc.scalar.activation(out=zr0[:], in_=xp4[:, 0:8, :, 0], func=Tanh,
                         scale=HALF_INV_R)
    hh0 = gt_pool.tile([P, KO, BL], f32, tag="hh")
    if has_gb:
        # hh = relu(xh + r*brec) ; r = (r'+1)/2
        nc.vector.scalar_tensor_tensor(out=hh0[:], in0=zr0[:, 4:8], scalar=1.0,
                                       in1=brecH_h[:], op0=Add, op1=Mult)
        nc.vector.tensor_add(out=hh0[:], in0=hh0[:], in1=xp4[:, 8:12, :, 0])
        nc.vector.tensor_scalar_max(hh0[:], hh0[:], 0.0)
    else:
        nc.vector.tensor_scalar_max(hh0[:], xp4[:, 8:12, :, 0], 0.0)
    f0 = gt_pool.tile([P, KO, BL], f32, tag="f")
    nc.vector.scalar_tensor_tensor(out=f0[:], in0=zr0[:, 0:4], scalar=1.0,
                                   in1=hh0[:], op0=Sub, op1=Mult)
    nc.vector.tensor_scalar_mul(hs4[:, :, :, 0], f0[:], -0.5)

    chunk_of_step = {}
    for c in range(NCH):
        chunk_of_step[CH_STARTS[c] + CHS[c] - 1] = c

    for t in range(1, T):
        if t % XS == XS - 10 and t // XS + 1 < XC:
            c = t // XS + 1
            emit_xproj_piece(c, 0, 6)
        if t % XS == XS - 5 and t // XS + 1 < XC:
            c = t // XS + 1
            emit_xproj_piece(c, 6, 6)

        # PE: one [P,32] accumulation group: ident inject (r+z), then r
        # chunks (cols 0:16), then z chunks (16:32).  tanh(r) reads the r
        # columns as soon as their last matmul lands (subtile deps) and
        # overlaps the z/h matmuls.
        zr_ps = ps_gru.tile([P, 8 * BL], f32, tag="zr_ps", name=f"zrp_{t}")
        h_ps = ps_gru.tile([P, KO * BL], f32, tag="h_ps", name=f"hp_{t}")
        xp_rz = xp4[:, 4:8, :, t], xp4[:, 0:4, :, t]
        nc.tensor.matmul(zr_ps[:, 0:16], lhsT=ident, rhs=xp_rz[0],
                         start=True, stop=False, skip_group_check=True)
        nc.tensor.matmul(zr_ps[:, 16:32], lhsT=ident, rhs=xp_rz[1],
                         start=False, stop=False, skip_group_check=True)
        for m in range(4):
            for ko in range(KO):
                nc.tensor.matmul(zr_ps[:, ds(BL * m, BL)],
                                 lhsT=Rb[:, ko, ts(4 + m, P)],
                                 rhs=hs4[:, ko, :, t - 1],
                                 start=False, stop=False, skip_group_check=True)
        for m in range(4):
            for ko in range(KO):
                nc.tensor.matmul(zr_ps[:, ds(16 + BL * m, BL)],
                                 lhsT=Rb[:, ko, ts(m, P)],
                                 rhs=hs4[:, ko, :, t - 1],
                                 start=False, stop=(ko == KO - 1) and (m == 3),
                                 skip_group_check=True)
        for m in range(4):
            for ko in range(KO):
                nc.tensor.matmul(h_ps[:, ds(BL * m, BL)],
                                 lhsT=Rb[:, ko, ts(8 + m, P)],
                                 rhs=hs4[:, ko, :, t - 1],
                                 start=(ko == 0), stop=(ko == KO - 1))
        # ACT: tanh(r) early (overlaps z/h matmuls), tanh(z) after
        rp_ = gt_pool.tile([P, 4, BL], f32, tag="tnh", name=f"rp_{t}")
        nc.scalar.activation(out=rp_[:],
                             in_=zr_ps[:, 0:16].rearrange("p (m b) -> p m b", b=BL),
                             func=Tanh, scale=HALF_INV_R)
        zp_ = gt_pool.tile([P, 4, BL], f32, tag="tnhz", name=f"zp_{t}")
        nc.scalar.activation(out=zp_[:],
                             in_=zr_ps[:, 16:32].rearrange("p (m b) -> p m b", b=BL),
                             func=Tanh, scale=HALF_INV_R)
        # DVE 5-op STT chain
        hp4 = h_ps.rearrange("p (m b) -> p m b", b=BL)
        u = gt_pool.tile([P, KO, BL], f32, tag="u", name=f"u_{t}")
        nc.vector.scalar_tensor_tensor(out=u[:], in0=rp_[:], scalar=1.0,
                                       in1=hp4, op0=Add, op1=Mult)
        w = gt_pool.tile([P, KO, BL], f32, tag="w", name=f"w_{t}")
        if has_gb:
            ub = gt_pool.tile([P, KO, BL], f32, tag="ub", name=f"ub_{t}")
            nc.vector.scalar_tensor_tensor(out=ub[:], in0=rp_[:], scalar=1.0,
                                           in1=brecH_h[:], op0=Add, op1=Mult)
            nc.vector.scalar_tensor_tensor(out=w[:], in0=u[:], scalar=U_SCALE,
                                           in1=ub[:], op0=Mult, op1=Add)
            nc.vector.tensor_add(out=w[:], in0=w[:], in1=xp4[:, 8:12, :, t])
        else:
            nc.vector.scalar_tensor_tensor(out=w[:], in0=u[:], scalar=U_SCALE,
                                           in1=xp4[:, 8:12, :, t], op0=Mult, op1=Add)
        dd = gt_pool.tile([P, KO, BL], f32, tag="d", name=f"d_{t}")
        nc.vector.scalar_tensor_tensor(out=dd[:], in0=w[:], scalar=0.0,
                                       in1=hs4[:, :, :, t - 1], op0=Max, op1=Sub)
        ff = gt_pool.tile([P, KO, BL], f32, tag="f", name=f"f_{t}")
        nc.vector.scalar_tensor_tensor(out=ff[:], in0=zp_[:], scalar=1.0,
                                       in1=dd[:], op0=Sub, op1=Mult)
        nc.vector.scalar_tensor_tensor(out=hs4[:, :, :, t], in0=ff[:], scalar=-0.5,
                                       in1=hs4[:, :, :, t - 1], op0=Mult, op1=Add)

        if t in chunk_of_step:
            chunk_tail(chunk_of_step[t], t + 1)
        pump(t)

    # ---------------- drain ----------------
    import os
    if os.environ.get("DBG_W3"):
        nc.sync.dma_start(out=ext["dbg4"], in_=w3b[:])
    if os.environ.get("DBG_HSEQ"):
        dbgf = gpool.tile([P, KO, NTOK], f32)
        nc.vector.tensor_copy(out=dbgf[:], in_=hseqT[:])
        nc.sync.dma_start(out=ext["dbg"], in_=dbgf[:])
    drain_all()
    if os.environ.get("DBG_H2G"):
        off = 0
        for c in range(NCH):
            n = NCORES * CHS[c] * BL
            nc.sync.dma_start(out=ext["dbg2"][:, :, ds(off, n)], in_=h2g8[c][:])
            off += n
    if rounds:
        push_round_scales(rounds[-1])
        drain_all()

    stack.close()


_BUILD_CACHE = {}


def _get_nc(has_b3: bool, has_gb: bool):
    key = (has_b3, has_gb)
    if key not in _BUILD_CACHE:
        _BUILD_CACHE[key] = _build(has_b3, has_gb)
    return _BUILD_CACHE[key]


def _make_in_maps(inputs):
    arrs = {k: np.ascontiguousarray(np.asarray(v, dtype=np.float32))
            for k, v in inputs.items()}
    in_maps = []
    for c in range(NCORES):
        in_maps.append({
            "encoder_input": arrs["encoder_input"][BL * c:BL * (c + 1)],
            "decoder_input": arrs["decoder_input"][BL * c:BL * (c + 1)],
            "gru_kernel": arrs["gru_kernel"],
            "gru_rec_kernel": arrs["gru_rec_kernel"],
            "gru_bias": arrs["gru_bias"],
            "w1": arrs["w1"], "b1": arrs["b1"],
            "w2": arrs["w2"], "b2": arrs["b2"],
            "w3": np.ascontiguousarray(arrs["w3"][:, VS * c:VS * (c + 1)]),
            "b3": np.ascontiguousarray(arrs["b3"][VS * c:VS * (c + 1)]),
        })
    flags = (bool(np.any(arrs["b3"])), bool(np.any(arrs["gru_bias"])))
    return in_maps, flags


def kernel(**inputs):
    global LAST_RESULT
    in_maps, (has_b3, has_gb) = _make_in_maps(inputs)
    nc = _get_nc(has_b3, has_gb)
    res = run_bass_kernel_spmd(nc, in_maps, core_ids=list(range(NCORES)),
                               trace=TRACE, **TRACE_KWARGS)
    LAST_RESULT = res
    full = np.empty((B, T, V), np.float32)
    for c in range(NCORES):
        full[:, :, VS * c:VS * (c + 1)] = \
            res.results[c]["out"].astype(np.float32).reshape(B, T, VS)
    return full
